# revision 102
# baseline (speedup 1.0000x reference)
"""Trainium2 Bass kernel for a single-head transformer layer (dense_transformer).

Reference math (fp32, unscaled single-head attention):
    Q = src@Wq+bq; K = src@Wk+bk; V = src@Wv+bv
    attn = softmax(Q@K^T) @ V @ Wo + bo
    x  = LN(src + attn)*g1 + be1
    out = LN(x + relu(x@W1+b1)@W2 + b2)*g2 + be2

Sharding: 8 cores = 4 batches x 2 sequence halves. Each core computes its
1024 query rows against the full 2048-token context of its batch (K/V work
duplicated 2x; no collectives). Host slices inputs / concatenates outputs.
srcT is column-PERMUTED per core so the core's own query half occupies
columns 0..1023 (attention is permutation-invariant over context order).

All DRAM tensors are host PRE-TILED so every DMA is a contiguous 4-32KB
run per partition (naive strided layouts generated 512B descriptors that
saturated the 16 DMA queues and starved the PE).

Attention path runs in float32r (tf32-like, full PE rate at N>=512),
except Q/K which are stored bf16 post-projection (the scores matmul is
bf16 x bf16; quantizing the final Q/K values costs only ~2e-4 extra
rel err while halving qT/kc SBUF, which funds deeper weight prefetch).
The feedforward runs in fp8 e4m3 with perf_mode=DoubleRow (2 fp8
weights/PE cell -> half the matmul instructions): W1/W2 are quantized
host-side with x16/x32 scaling so their +-1/32, +-1/64 ranges land in
e4m3's normal range; the exact inverse scales are applied in the relu
epilogue (scale=1/16) and the FF2 eviction (*1/32). x and h are cast to
e4m3 on the fly (unit-scale after LN). Softmax/P@V stay f32r.

Exact host-side folds (no accuracy cost): bk dropped (softmax-invariant),
bv -> residual as bv@Wo, bo -> srcq, LN1 affine -> W1'=diag(g1)@W1 and
b1'=b1+be1@W1 (kernel stores only the normalized x-hat; the residual
affine is rebuilt in phase 4's Vector slack), be1+b2 -> one vector.

Per-core plan:
    chunk pair (0,1) then (2,3); each pair loads Wk/Wv tiles ONCE; pair-1
    projections interleave between the pair-0 attends to hide their DMA:
        kc[e,kc] = Wk.T @ srcT_c ;  vc[kc,e] = srcT_c.T @ Wv
        (pair 0 also: qT[e,q] = Wq.T @ srcT_{0,1} (+bq), one Wq pass)
        attend(c): pc = exp(kc.T @ qT); aT += vc.T @ pc; sums += 1.T @ pc
    O[q,eo] = (aT.T @ Wo)*(1/sums) + (src+bo) ; x-hat = normalize(.) ;
    xT = x-hat.T (PE transposes, fp8 eviction)
    FF chunk groups [2,6], one PSUM accumulation per group:
        hT = relu8(W1c.T @ xT * 1/16 + b1') ; PSUM += hT.T @ W2c ;
        evict once per group (x += PSUM/32).  Last-group evictions emit
        sum(x2) via accum_out and a Scalar Square pass emits sum(x2^2),
        so LN2 needs no Vector bn_stats; LN2 + store run per row inside
        the last group's matmul window.
"""

import numpy as np
from contextlib import ExitStack

import concourse.bacc as bacc
import concourse.tile as tile
from concourse import mybir
from concourse.masks import make_identity

P = 128
E = 1024          # embed
F = 4096          # dff
S = 2048          # context length per batch
NQ = 1024         # query rows per core
ET = E // P       # 8
FT = F // P       # 32
QS = NQ // 512    # 2 query slices of 512
KCH = 512         # k-chunk size
NCH = S // KCH    # 4 chunks
KT = KCH // P     # 4 k-tiles per chunk
FCH = 4           # f-tiles per FF chunk (512 f-columns)
NFC = FT // FCH   # 8 FF chunks
W1_SCALE = 16.0   # host premultiplier on W1 before e4m3 cast
W2_SCALE = 32.0   # host premultiplier on W2 before e4m3 cast
f32 = mybir.dt.float32
f32r = mybir.dt.float32r
bf16 = mybir.dt.bfloat16
f8 = mybir.dt.float8e4
u8 = mybir.dt.uint8
DR = mybir.MatmulPerfMode.DoubleRow
EPS = 1e-5

SUB = mybir.AluOpType.subtract
MULT = mybir.AluOpType.mult
ADD = mybir.AluOpType.add


def _mm(ap):
    """Bitcast matmul operands/producers to float32r (full PE rate at
    N>=256). The BIR verifier requires every fp32r matmul operand to be
    *produced* as fp32r, so the same bitcast is applied to the producing
    DMA (both sides) or ACT/DVE eviction output."""
    return ap.bitcast(f32r)


def _f8(ap):
    """uint8 DRAM bytes -> fp8e4m3 view (numpy has no fp8 dtype)."""
    return ap.bitcast(f8)


def build_program():
    nc = bacc.Bacc("TRN2", target_bir_lowering=False, debug=False, num_devices=8)

    srcTt = nc.dram_tensor("srcTt", [P, NCH, ET, KCH], bf16, kind="ExternalInput").ap()
    srcq = nc.dram_tensor("srcq", [NQ, E], f32, kind="ExternalInput").ap()
    Wqt = nc.dram_tensor("Wqt", [P, ET, ET, P], bf16, kind="ExternalInput").ap()
    Wkt = nc.dram_tensor("Wkt", [P, ET, ET, P], bf16, kind="ExternalInput").ap()
    Wvt = nc.dram_tensor("Wvt", [P, 2, ET, 512], bf16, kind="ExternalInput").ap()
    Wot = nc.dram_tensor("Wot", [P, ET, E], bf16, kind="ExternalInput").ap()
    W1t = nc.dram_tensor("W1t", [P, NFC, ET, 512], u8, kind="ExternalInput").ap()
    W2t = nc.dram_tensor("W2t", [P, NFC, FCH, E], u8, kind="ExternalInput").ap()
    # bq | b1 pre-tiled host-side to [P, ET+FT] (a naive "(t p) -> p t"
    # rearrange DMA costs ~1k 4-byte descriptors).  bk is dropped: it only
    # adds a per-query constant to the logits, which softmax cancels.  bv
    # is folded into the residual host-side as bv@Wo (softmax weights sum
    # to 1).  The LN1 affine is folded into the FF weights host-side
    # (W1' = diag(g1) @ W1, b1' = b1 + be1 @ W1), so the kernel only
    # normalizes x and the transposed copy xT stays affine-free.
    bqb1 = nc.dram_tensor("bqb1", [P, ET + FT], f32, kind="ExternalInput").ap()
    beb2 = nc.dram_tensor("beb2", [E], f32, kind="ExternalInput").ap()
    g1 = nc.dram_tensor("g1", [E], f32, kind="ExternalInput").ap()
    g2 = nc.dram_tensor("g2", [E], f32, kind="ExternalInput").ap()
    be2 = nc.dram_tensor("be2", [E], f32, kind="ExternalInput").ap()
    out = nc.dram_tensor("out", [NQ, E], f32, kind="ExternalOutput").ap()

    with tile.TileContext(nc) as tc, ExitStack() as ctx:
        consts = ctx.enter_context(tc.tile_pool(name="consts", bufs=1))
        lnp = ctx.enter_context(tc.tile_pool(name="lnp", bufs=4))

        def bcast(vec, n, pool):
            t = pool.tile([P, n], f32, tag=f"bc_{vec.tensor.name}")
            nc.sync.dma_start(out=t, in_=vec.partition_broadcast(P))
            return t

        def ln_stats(t):
            """mean/rstd of t along the free dim -> (mv, rstd)."""
            stats = lnp.tile([P, 2, 6], f32, tag="stats")
            for sg in range(2):
                nc.vector.bn_stats(out=stats[:, sg, :], in_=t[:, sg * 512:(sg + 1) * 512])
            mv = lnp.tile([P, 2], f32, tag="mv")
            nc.vector.bn_aggr(out=mv, in_=stats)
            rstd = lnp.tile([P, 1], f32, tag="rstd")
            nc.scalar.activation(out=rstd, in_=mv[:, 1:2],
                                 func=mybir.ActivationFunctionType.Sqrt,
                                 bias=eps_sb, scale=1.0)
            nc.vector.reciprocal(out=rstd, in_=rstd)
            return mv, rstd

        def ln_normalize(t):
            """t = (t - mu) * rstd, one fused Vector op (no affine)."""
            mv, rstd = ln_stats(t)
            nc.vector.tensor_scalar(out=t, in0=t, scalar1=mv[:, 0:1],
                                    scalar2=rstd, op0=SUB, op1=MULT)

        def layernorm_inplace(t, g_bc, be_bc):
            """t: [P, E] SBUF tile; LN along free dim, then *g + be."""
            mv, rstd = ln_stats(t)
            nc.vector.scalar_tensor_tensor(out=t, in0=t, scalar=mv[:, 0:1],
                                           in1=g_bc, op0=SUB, op1=MULT)
            nc.vector.scalar_tensor_tensor(out=t, in0=t, scalar=rstd,
                                           in1=be_bc, op0=MULT, op1=ADD)

        stA = ExitStack()   # aT lives through phase 3
        late = ExitStack()  # pools allocated after phase 2 frees its SBUF
        try:
            aT_pool = stA.enter_context(tc.tile_pool(name="aT_pool", bufs=1))
            aT = aT_pool.tile([P, ET, NQ], bf16)

            # ------------- Phase 1+2: QKV projections + attention -------------
            with ExitStack() as ph2:
                qT_pool = ph2.enter_context(tc.tile_pool(name="qT_pool", bufs=1))
                qT = qT_pool.tile([P, ET, NQ], bf16)
                stpc_pool = ph2.enter_context(tc.tile_pool(name="stpc", bufs=3))
                kc_pool = ph2.enter_context(tc.tile_pool(name="kcp", bufs=2))
                vc_pool = ph2.enter_context(tc.tile_pool(name="vcp", bufs=2))
                wk_pool = ph2.enter_context(tc.tile_pool(name="wk", bufs=10))
                wv_pool = ph2.enter_context(tc.tile_pool(name="wv", bufs=2))
                ps_kv = ph2.enter_context(tc.tile_pool(name="ps_kv", bufs=2, space="PSUM"))
                ps_s = ph2.enter_context(tc.tile_pool(name="ps_s", bufs=2, space="PSUM"))
                ps_a = ph2.enter_context(tc.tile_pool(name="ps_a", bufs=2, space="PSUM"))
                ps_sum = ph2.enter_context(tc.tile_pool(name="ps_sum", bufs=1, space="PSUM"))

                # per-partition bias layouts: element i lands at [i % 128, i // 128]
                bias_sb = consts.tile([P, ET + FT], f32)
                nc.sync.dma_start(out=bias_sb, in_=bqb1)
                bq_sb = bias_sb[:, 0:ET]
                b1_sb = bias_sb[:, ET:]

                ones0 = consts.tile([P, 1], f32)
                nc.vector.memset(ones0, 1.0)
                ones_sb = consts.tile([P, 1], f32)
                nc.vector.tensor_copy(out=_mm(ones_sb), in_=ones0)
                eps_sb = consts.tile([P, 1], f32)
                nc.vector.memset(eps_sb, EPS)
                inv_w2s = consts.tile([P, 1], f32)
                nc.vector.memset(inv_w2s, 1.0 / W2_SCALE)

                sums = []
                for qs in range(QS):
                    sums_t = ps_sum.tile([1, 512], f32, tag=f"sums{qs}", name=f"sums{qs}")
                    sums.append(sums_t)

                kcs, vcs = {}, {}

                def attend(cc, first, last):
                    """S^T -> exp -> sums and aT accumulation for chunk cc."""
                    kc, vc = kcs.pop(cc), vcs.pop(cc)
                    pc = stpc_pool.tile([P, KT, NQ], f32, tag="stpc", name=f"pc{cc}")
                    for kt in range(KT):
                        for qs in range(QS):
                            ps = ps_s.tile([P, 512], f32, tag="ps")
                            for e_t in range(ET):
                                nc.tensor.matmul(ps, kc[:, e_t, kt * P:(kt + 1) * P],
                                                 qT[:, e_t, qs * 512:(qs + 1) * 512],
                                                 start=(e_t == 0), stop=(e_t == ET - 1))
                            nc.scalar.activation(out=_mm(pc[:, kt, qs * 512:(qs + 1) * 512]),
                                                 in_=ps,
                                                 func=mybir.ActivationFunctionType.Exp)
                            nc.tensor.matmul(sums[qs], _mm(ones_sb),
                                             _mm(pc[:, kt, qs * 512:(qs + 1) * 512]),
                                             start=(first and kt == 0),
                                             stop=(last and kt == KT - 1))
                    # aT += vc.T @ pc
                    for qs in range(QS):
                        for e_t in range(ET):
                            ps = ps_a.tile([P, 512], f32, tag="ps")
                            for kt in range(KT):
                                nc.tensor.matmul(ps, _mm(vc[:, kt, e_t * P:(e_t + 1) * P]),
                                                 _mm(pc[:, kt, qs * 512:(qs + 1) * 512]),
                                                 start=(kt == 0), stop=(kt == KT - 1))
                            dst = aT[:, e_t, qs * 512:(qs + 1) * 512]
                            if first:
                                nc.vector.tensor_copy(out=dst, in_=ps)
                            else:
                                nc.vector.tensor_add(out=dst, in0=dst, in1=ps)

                sts = {}

                def st_load(cc, split=False):
                    st = stpc_pool.tile([P, ET, KCH], bf16, tag="stpc", name=f"st{cc}")
                    if split:
                        # per-d_t transfers: the first projection matmul only
                        # waits on slice 0, not the whole 2MB chunk
                        for d_t in range(ET):
                            nc.sync.dma_start(out=st[:, d_t, :],
                                              in_=srcTt[:, cc, d_t, :])
                    else:
                        nc.sync.dma_start(out=st, in_=srcTt[:, cc])
                    sts[cc] = st

                def kproj(pr, c0):
                    """K^T chunks [e, kc] for the pair, one Wk tile pass."""
                    for cc in (c0, c0 + 1):
                        kcs[cc] = kc_pool.tile([P, ET, KCH], bf16, tag="kc", name=f"kc{cc}")
                    for e_t in range(ET):
                        wk_t = wk_pool.tile([P, ET, P], bf16, tag="wk", name=f"wk{pr}_{e_t}")
                        nc.sync.dma_start(out=wk_t, in_=Wkt[:, e_t])
                        for cc in (c0, c0 + 1):
                            ps = ps_kv.tile([P, KCH], f32, tag="ps")
                            for d_t in range(ET):
                                nc.tensor.matmul(ps, wk_t[:, d_t, :],
                                                 sts[cc][:, d_t, :],
                                                 start=(d_t == 0), stop=(d_t == ET - 1))
                            nc.vector.tensor_copy(out=kcs[cc][:, e_t, :], in_=ps)

                def vproj(pr, c0):
                    """V chunks [kc, e] for the pair, one Wv tile pass."""
                    for cc in (c0, c0 + 1):
                        vcs[cc] = vc_pool.tile([P, KT, E], f32, tag="vc", name=f"vc{cc}")
                    for es in range(2):
                        wv_t = wv_pool.tile([P, ET, 512], bf16, tag="wv", name=f"wv{pr}_{es}")
                        nc.sync.dma_start(out=wv_t, in_=Wvt[:, es])
                        for cc in (c0, c0 + 1):
                            for kt in range(KT):
                                ps = ps_kv.tile([P, 512], f32, tag="ps")
                                for d_t in range(ET):
                                    nc.tensor.matmul(
                                        ps, sts[cc][:, d_t, kt * P:(kt + 1) * P],
                                        wv_t[:, d_t, :],
                                        start=(d_t == 0), stop=(d_t == ET - 1))
                                nc.vector.tensor_copy(
                                    out=_mm(vcs[cc][:, kt, es * 512:(es + 1) * 512]),
                                    in_=ps)

                # -- pair 0, startup-critical DMA order: wq0-slice, st0,
                # wq0-rest, wq1..7, st1 -- all Wq is in flight before st1 so
                # the Q matmul stream never outruns its weights
                wq_ts = [wk_pool.tile([P, ET, P], bf16, tag="wk", name="wq0")]
                nc.sync.dma_start(out=wq_ts[0][:, 0, :], in_=Wqt[:, 0, 0, :])
                st_load(0, split=True)
                for d_t in range(1, ET):
                    nc.sync.dma_start(out=wq_ts[0][:, d_t, :],
                                      in_=Wqt[:, 0, d_t, :])
                for e_t in range(1, ET):
                    nxt = wk_pool.tile([P, ET, P], bf16, tag="wk", name=f"wq{e_t}")
                    nc.sync.dma_start(out=nxt, in_=Wqt[:, e_t])
                    wq_ts.append(nxt)
                st_load(1, split=True)
                # Q projection for both query slices (srcT is permuted so
                # chunks 0-1 ARE the core's query rows); one Wq tile pass.
                # (e_t=0, qs=1) is deferred one iteration so the first matmuls
                # only need st0 -- st1 is still streaming in at that point.
                def qproj(e_t, qs):
                    ps = ps_kv.tile([P, 512], f32, tag="ps")
                    for d_t in range(ET):
                        nc.tensor.matmul(ps, wq_ts[e_t][:, d_t, :],
                                         sts[qs][:, d_t, :],
                                         start=(d_t == 0), stop=(d_t == ET - 1))
                    nc.vector.tensor_scalar_add(
                        out=qT[:, e_t, qs * 512:(qs + 1) * 512],
                        in0=ps, scalar1=bq_sb[:, e_t:e_t + 1])

                for e_t in range(ET):
                    qproj(e_t, 0)
                    if e_t == 1:
                        qproj(0, 1)
                    if e_t >= 1:
                        qproj(e_t, 1)
                kproj(0, 0)
                vproj(0, 0)
                # interleave pair-1 DMAs/projections between the pair-0
                # attends so pair-1's weight traffic hides under compute
                attend(0, first=True, last=False)
                st_load(2)
                st_load(3)
                kproj(1, 2)
                attend(1, first=False, last=False)
                vproj(1, 2)
                attend(2, first=False, last=False)
                attend(3, first=False, last=True)
                del sts

                # softmax denominators: spread sums[1, q] across partitions
                # via K=1 matmuls (1-partition DMAs fail NEFF load)
                sums_sb = consts.tile([1, NQ], f32)
                for qs in range(QS):
                    nc.vector.tensor_copy(out=sums_sb[:, qs * 512:(qs + 1) * 512],
                                          in_=sums[qs])
                one_sp = consts.tile([1, 1], f32)
                nc.vector.memset(one_sp, 1.0)
                rsum = consts.tile([P, ET], f32)
                for t in range(ET):
                    pst = ps_kv.tile([P, 1], f32, tag="ps", name=f"spread{t}")
                    nc.tensor.matmul(pst, sums_sb[0:1, t * P:(t + 1) * P], one_sp,
                                     start=True, stop=True)
                    nc.vector.tensor_copy(out=rsum[:, t:t + 1], in_=pst)
                nc.vector.reciprocal(out=rsum, in_=rsum)

            # ------------- Phase 3: O, residual, LN1, transpose -------------
            # wo first: it lands in the earliest-freed phase-2 SBUF, so its
            # DMA can start before the attention tail fully drains
            wo_pool = late.enter_context(tc.tile_pool(name="wo", bufs=1))
            x_pool = late.enter_context(tc.tile_pool(name="x_pool", bufs=1))
            xT_pool = late.enter_context(tc.tile_pool(name="xT_pool", bufs=1))
            wc_pool = late.enter_context(tc.tile_pool(name="wc", bufs=16))
            bcp = late.enter_context(tc.tile_pool(name="bcp", bufs=1))
            x_sb = x_pool.tile([P, ET, E], f32)   # [q(8x128), e]
            xT = xT_pool.tile([P, ET, NQ], f8)    # [e, q] fp8 for the FF

            g1_bc = bcast(g1, E, bcp)
            beb2_bc = bcast(beb2, E, bcp)   # be1 + b2, combined host-side
            ident = bcp.tile([P, P], f32, tag="ident")
            make_identity(nc, ident)

            # per-tile DMAs so the first O matmul only waits on wo[0]
            wo_sb = wo_pool.tile([P, ET, E], bf16)
            for e_t in range(ET):
                nc.sync.dma_start(out=wo_sb[:, e_t, :], in_=Wot[:, e_t])

            with ExitStack() as ph3:
                sq2_pool = ph3.enter_context(tc.tile_pool(name="sq2", bufs=3))
                ps_o = ph3.enter_context(tc.tile_pool(name="ps_o", bufs=6, space="PSUM"))
                ps_t = ph3.enter_context(tc.tile_pool(name="ps_t", bufs=2, space="PSUM"))

                # prefetch first FF chunk weights during phase 3
                w1c0 = wc_pool.tile([P, ET, 512], f8, tag="wc", name="w1c0")
                nc.sync.dma_start(out=w1c0, in_=_f8(W1t[:, 0]))
                w2c0 = wc_pool.tile([P, FCH, E], f8, tag="wc", name="w2c0")
                nc.sync.dma_start(out=w2c0, in_=_f8(W2t[:, 0]))

                for q_t in range(ET):
                    sq = sq2_pool.tile([P, E], f32, tag="sq")
                    nc.sync.dma_start(out=sq, in_=srcq[q_t * P:(q_t + 1) * P, :])
                    xt_row = x_sb[:, q_t, :]
                    for eo in range(2):
                        ps = ps_o.tile([P, 512], f32, tag="ps")
                        for e_t in range(ET):
                            nc.tensor.matmul(ps, aT[:, e_t, q_t * P:(q_t + 1) * P],
                                             wo_sb[:, e_t, eo * 512:(eo + 1) * 512],
                                             start=(e_t == 0), stop=(e_t == ET - 1))
                        # x = O*rsum + (src+bo), one fused Vector op
                        # (bo and bv@Wo are folded into srcq on the host)
                        nc.vector.scalar_tensor_tensor(
                            out=x_sb[:, q_t, eo * 512:(eo + 1) * 512], in0=ps,
                            scalar=rsum[:, q_t:q_t + 1],
                            in1=sq[:, eo * 512:(eo + 1) * 512], op0=MULT, op1=ADD)
                    # normalize only -- the g1/be1 affine rides the transpose
                    # eviction (per-partition in the transposed domain)
                    ln_normalize(xt_row)
                    for e_t in range(ET):
                        pst = ps_t.tile([P, P], f32, tag="ps")
                        nc.tensor.transpose(pst, x_sb[:, q_t, e_t * P:(e_t + 1) * P], ident)
                        nc.scalar.activation(out=xT[:, e_t, q_t * P:(q_t + 1) * P],
                                             in_=pst,
                                             func=mybir.ActivationFunctionType.Copy)

            # ------------- Phase 4: feedforward (fp8 DoubleRow) + LN2 -------
            g2_bc = bcast(g2, E, bcp)
            be2_bc = bcast(be2, E, bcp)
            with ExitStack() as ph4:
                # x_sb holds normalized-but-unaffined LN1 output; rebuild the
                # residual base x*g1 + (be1+b2) here, in group-0's Vector slack
                for q_t in range(ET):
                    row = x_sb[:, q_t, :]
                    nc.vector.tensor_mul(out=row, in0=row, in1=g1_bc)
                    nc.vector.tensor_add(out=row, in0=row, in1=beb2_bc)
                hc_pool = ph4.enter_context(tc.tile_pool(name="hc", bufs=7))
                sqs_pool = ph4.enter_context(tc.tile_pool(name="sqs", bufs=2))
                ps_h = ph4.enter_context(tc.tile_pool(name="ps_h", bufs=2, space="PSUM"))
                ps_f = ph4.enter_context(tc.tile_pool(name="ps_f", bufs=6, space="PSUM"))

                # FF chunk groups: one PSUM accumulation per group; the
                # large last group gives the inline LN2 a window to hide in
                GRPS = [1, 7]
                for g, GRP in enumerate(GRPS):
                    gbase = sum(GRPS[:g])
                    w1cs, w2cs, hTcs = [], [], []
                    for h in range(GRP):
                        fc = gbase + h
                        if fc == 0:
                            w1c = w1c0
                        else:
                            w1c = wc_pool.tile([P, ET, 512], f8, tag="wc", name=f"w1c{fc}")
                            nc.sync.dma_start(out=w1c, in_=_f8(W1t[:, fc]))
                        hTc = hc_pool.tile([P, FCH, NQ], f8, tag="hc", name=f"hc{fc}")
                        w1cs.append(w1c)
                        hTcs.append(hTc)
                        for fl in range(FCH):
                            f_t = fc * FCH + fl
                            for qs in range(QS):
                                ps = ps_h.tile([P, 512], f32, tag="ps")
                                for dp in range(ET // 2):
                                    nc.tensor.matmul(
                                        ps, w1c[:, 2 * dp:2 * dp + 2, fl * P:(fl + 1) * P],
                                        xT[:, 2 * dp:2 * dp + 2, qs * 512:(qs + 1) * 512],
                                        start=(dp == 0), stop=(dp == ET // 2 - 1),
                                        perf_mode=DR)
                                # h = relu(ps/W1_SCALE + b1), cast to fp8
                                nc.scalar.activation(
                                    out=hTc[:, fl, qs * 512:(qs + 1) * 512], in_=ps,
                                    func=mybir.ActivationFunctionType.Relu,
                                    bias=b1_sb[:, f_t:f_t + 1], scale=1.0 / W1_SCALE)

                        fc2 = gbase + h
                        if fc2 == 0:
                            w2cs.append(w2c0)
                        else:
                            w2c = wc_pool.tile([P, FCH, E], f8, tag="wc", name=f"w2c{fc2}")
                            nc.sync.dma_start(out=w2c, in_=_f8(W2t[:, fc2]))
                            w2cs.append(w2c)

                    last = g == len(GRPS) - 1
                    for q_t in range(ET):
                        asum = (lnp.tile([P, 4], f32, tag="asum", name=f"asum{q_t}")
                                if last else None)
                        for eo in range(2):
                            ps = ps_f.tile([P, 512], f32, tag="ps")
                            for h in range(GRP):
                                for fp_ in range(FCH // 2):
                                    nc.tensor.matmul(
                                        ps,
                                        hTcs[h][:, 2 * fp_:2 * fp_ + 2, q_t * P:(q_t + 1) * P],
                                        w2cs[h][:, 2 * fp_:2 * fp_ + 2, eo * 512:(eo + 1) * 512],
                                        start=(h == 0 and fp_ == 0),
                                        stop=(h == GRP - 1 and fp_ == FCH // 2 - 1),
                                        perf_mode=DR)
                            dst = x_sb[:, q_t, eo * 512:(eo + 1) * 512]
                            # x += ps/W2_SCALE; in the last group the same op
                            # emits sum(x2) per half as LN2's mean input
                            nc.vector.scalar_tensor_tensor(
                                out=dst, in0=ps, scalar=inv_w2s, in1=dst,
                                op0=MULT, op1=ADD,
                                accum_out=asum[:, eo:eo + 1] if last else None)
                            if last:
                                # sum(x2^2) via an idle-Scalar Square pass
                                sqscr = sqs_pool.tile([P, 512], f32, tag="sq")
                                nc.scalar.activation(
                                    out=sqscr, in_=dst,
                                    func=mybir.ActivationFunctionType.Square,
                                    accum_out=asum[:, 2 + eo:3 + eo])
                        if last:
                            # final row: LN2 from the accumulated sums + store
                            # -- overlaps the remaining rows' matmuls
                            row = x_sb[:, q_t, :]
                            mu = lnp.tile([P, 1], f32, tag="mu1")
                            nc.vector.tensor_add(out=mu, in0=asum[:, 0:1],
                                                 in1=asum[:, 1:2])
                            nc.vector.tensor_scalar_mul(out=mu, in0=mu,
                                                        scalar1=1.0 / E)
                            vr = lnp.tile([P, 1], f32, tag="vr")
                            nc.vector.tensor_add(out=vr, in0=asum[:, 2:3],
                                                 in1=asum[:, 3:4])
                            mu2 = lnp.tile([P, 1], f32, tag="mu2")
                            nc.vector.tensor_mul(out=mu2, in0=mu, in1=mu)
                            nc.vector.tensor_scalar(out=vr, in0=vr,
                                                    scalar1=1.0 / E, scalar2=mu2,
                                                    op0=MULT, op1=SUB)
                            rstd = lnp.tile([P, 1], f32, tag="rstd2")
                            nc.scalar.activation(
                                out=rstd, in_=vr,
                                func=mybir.ActivationFunctionType.Sqrt,
                                bias=eps_sb, scale=1.0)
                            nc.vector.reciprocal(out=rstd, in_=rstd)
                            nc.vector.scalar_tensor_tensor(out=row, in0=row,
                                                           scalar=mu, in1=g2_bc,
                                                           op0=SUB, op1=MULT)
                            nc.vector.scalar_tensor_tensor(out=row, in0=row,
                                                           scalar=rstd, in1=be2_bc,
                                                           op0=MULT, op1=ADD)
                            nc.sync.dma_start(out=out[q_t * P:(q_t + 1) * P, :], in_=row)
        finally:
            # LIFO: `late` pools were entered after stA, so release them first
            late.close()
            stA.close()

    nc.compile()
    return nc


_NC_CACHE = None


def _pretile(inputs):
    """Host-side re-layouts so every DMA is contiguous per partition.
    W1/W2 are scaled and quantized to fp8 e4m3 (passed as uint8 bytes)."""
    import ml_dtypes
    e4m3 = ml_dtypes.float8_e4m3fn
    Wq = np.asarray(inputs["Wq"], np.float32)
    Wk = np.asarray(inputs["Wk"], np.float32)
    Wv = np.asarray(inputs["Wv"], np.float32)
    Wo = np.asarray(inputs["Wo"], np.float32)
    # fold the LN1 affine into the first FF layer (exact):
    #   (g1*x^ + be1) @ W1 + b1  ==  x^ @ (diag(g1) @ W1) + (b1 + be1 @ W1)
    W1raw = np.asarray(inputs["W1"], np.float32)
    g1v = np.asarray(inputs["g1"], np.float32)
    be1v = np.asarray(inputs["be1"], np.float32)
    W1 = (g1v[:, None] * W1raw) * W1_SCALE
    b1f = np.asarray(inputs["b1"], np.float32) + be1v @ W1raw
    W2 = np.asarray(inputs["W2"], np.float32) * W2_SCALE
    c = np.ascontiguousarray
    bf = ml_dtypes.bfloat16
    d = {
        "Wqt": c(Wq.reshape(ET, P, ET, P).transpose(1, 2, 0, 3).astype(bf)),
        "Wkt": c(Wk.reshape(ET, P, ET, P).transpose(1, 2, 0, 3).astype(bf)),
        "Wvt": c(Wv.reshape(ET, P, 2, 512).transpose(1, 2, 0, 3).astype(bf)),
        "Wot": c(Wo.reshape(ET, P, E).transpose(1, 0, 2).astype(bf)),
        "W1t": c(W1.reshape(ET, P, NFC, 512).transpose(1, 2, 0, 3)
                 .astype(e4m3)).view(np.uint8),
        "W2t": c(W2.reshape(NFC, FCH, P, E).transpose(2, 0, 1, 3)
                 .astype(e4m3)).view(np.uint8),
        "bqb1": c(np.concatenate(
            [np.asarray(inputs["bq"], np.float32).reshape(ET, P).T,
             b1f.reshape(FT, P).T], axis=1)),
        "beb2": c(be1v + np.asarray(inputs["b2"], np.float32)),
    }
    for n in ["g1", "g2", "be2"]:
        d[n] = c(np.asarray(inputs[n], np.float32))
    return d


def make_in_maps(inputs):
    src = np.ascontiguousarray(np.asarray(inputs["src"], dtype=np.float32))
    # residual offset: out-projection bias + bv routed through Wo
    # (attention weights sum to 1, so  softmax(S) @ (V0 + bv) @ Wo + bo
    #  == softmax(S) @ V0 @ Wo + (bv @ Wo + bo))
    bo = (np.asarray(inputs["bo"], np.float32)
          + np.asarray(inputs["bv"], np.float32)
          @ np.asarray(inputs["Wo"], np.float32))
    shared = _pretile(inputs)

    in_maps = []
    for core in range(8):
        b, h = core // 2, core % 2
        src_b = src[b]                        # [2048, 1024]
        # permute context so this core's query half is columns 0..1023
        perm = np.concatenate([src_b[h * NQ:(h + 1) * NQ, :],
                               src_b[(1 - h) * NQ:(2 - h) * NQ, :]])
        srcT = perm.T                         # [1024, 2048]
        # [p, c, t, k] = srcT[t*128+p, c*512+k]; bf16 on the wire (the
        # projection matmuls consume it in bf16)
        import ml_dtypes
        srcTt = np.ascontiguousarray(
            srcT.reshape(ET, P, NCH, KCH).transpose(1, 2, 0, 3)
            .astype(ml_dtypes.bfloat16))
        # bo (the out-projection bias) is folded into the residual here
        srcq = np.ascontiguousarray(src_b[h * NQ:(h + 1) * NQ, :] + bo)
        in_maps.append({"srcTt": srcTt, "srcq": srcq, **shared})
    return in_maps


def gather_out(results):
    out = np.empty((4, S, E), np.float32)
    for core in range(8):
        b, h = core // 2, core % 2
        out[b, h * NQ:(h + 1) * NQ, :] = results[core]["out"]
    return out


def kernel(**inputs):
    global _NC_CACHE
    from concourse.bass_utils import run_bass_kernel_spmd

    in_maps = make_in_maps(inputs)
    if _NC_CACHE is None:
        _NC_CACHE = build_program()
    res = run_bass_kernel_spmd(_NC_CACHE, in_maps, list(range(8)))
    return gather_out(res.results)


if __name__ == "__main__":
    nc = build_program()
    print("build + compile OK")


# revision 103
# speedup vs baseline: 1.0048x; 1.0048x over previous
"""Trainium2 Bass kernel for a single-head transformer layer (dense_transformer).

Reference math (fp32, unscaled single-head attention):
    Q = src@Wq+bq; K = src@Wk+bk; V = src@Wv+bv
    attn = softmax(Q@K^T) @ V @ Wo + bo
    x  = LN(src + attn)*g1 + be1
    out = LN(x + relu(x@W1+b1)@W2 + b2)*g2 + be2

Sharding: 8 cores = 4 batches x 2 sequence halves. Each core computes its
1024 query rows against the full 2048-token context of its batch (K/V work
duplicated 2x; no collectives). Host slices inputs / concatenates outputs.
srcT is column-PERMUTED per core so the core's own query half occupies
columns 0..1023 (attention is permutation-invariant over context order).

All DRAM tensors are host PRE-TILED so every DMA is a contiguous 4-32KB
run per partition (naive strided layouts generated 512B descriptors that
saturated the 16 DMA queues and starved the PE).

Attention path runs in float32r (tf32-like, full PE rate at N>=512),
except Q/K which are stored bf16 post-projection (the scores matmul is
bf16 x bf16; quantizing the final Q/K values costs only ~2e-4 extra
rel err while halving qT/kc SBUF, which funds deeper weight prefetch).
The feedforward runs in fp8 e4m3 with perf_mode=DoubleRow (2 fp8
weights/PE cell -> half the matmul instructions): W1/W2 are quantized
host-side with x16/x32 scaling so their +-1/32, +-1/64 ranges land in
e4m3's normal range; the exact inverse scales are applied in the relu
epilogue (scale=1/16) and the FF2 eviction (*1/32). x and h are cast to
e4m3 on the fly (unit-scale after LN). Softmax/P@V stay f32r.

Exact host-side folds (no accuracy cost): bk dropped (softmax-invariant),
bv -> residual as bv@Wo, bo -> srcq, LN1 affine -> W1'=diag(g1)@W1 and
b1'=b1+be1@W1 (kernel stores only the normalized x-hat; the residual
affine is rebuilt in phase 4's Vector slack), be1+b2 -> one vector.

Per-core plan:
    chunk pair (0,1) then (2,3); each pair loads Wk/Wv tiles ONCE; pair-1
    projections interleave between the pair-0 attends to hide their DMA:
        kc[e,kc] = Wk.T @ srcT_c ;  vc[kc,e] = srcT_c.T @ Wv
        (pair 0 also: qT[e,q] = Wq.T @ srcT_{0,1} (+bq), one Wq pass)
        attend(c): pc = exp(kc.T @ qT); aT += vc.T @ pc; sums += 1.T @ pc
    O[q,eo] = (aT.T @ Wo)*(1/sums) + (src+bo) ; x-hat = normalize(.) ;
    xT = x-hat.T (PE transposes, fp8 eviction)
    FF chunk groups [2,6], one PSUM accumulation per group:
        hT = relu8(W1c.T @ xT * 1/16 + b1') ; PSUM += hT.T @ W2c ;
        evict once per group (x += PSUM/32).  Last-group evictions emit
        sum(x2) via accum_out and a Scalar Square pass emits sum(x2^2),
        so LN2 needs no Vector bn_stats; LN2 + store run per row inside
        the last group's matmul window.
"""

import numpy as np
from contextlib import ExitStack

import concourse.bacc as bacc
import concourse.tile as tile
from concourse import mybir
from concourse.masks import make_identity

P = 128
E = 1024          # embed
F = 4096          # dff
S = 2048          # context length per batch
NQ = 1024         # query rows per core
ET = E // P       # 8
FT = F // P       # 32
QS = NQ // 512    # 2 query slices of 512
KCH = 512         # k-chunk size
NCH = S // KCH    # 4 chunks
KT = KCH // P     # 4 k-tiles per chunk
FCH = 4           # f-tiles per FF chunk (512 f-columns)
NFC = FT // FCH   # 8 FF chunks
W1_SCALE = 16.0   # host premultiplier on W1 before e4m3 cast
W2_SCALE = 32.0   # host premultiplier on W2 before e4m3 cast
f32 = mybir.dt.float32
f32r = mybir.dt.float32r
bf16 = mybir.dt.bfloat16
f8 = mybir.dt.float8e4
u8 = mybir.dt.uint8
DR = mybir.MatmulPerfMode.DoubleRow
EPS = 1e-5

SUB = mybir.AluOpType.subtract
MULT = mybir.AluOpType.mult
ADD = mybir.AluOpType.add


def _mm(ap):
    """Bitcast matmul operands/producers to float32r (full PE rate at
    N>=256). The BIR verifier requires every fp32r matmul operand to be
    *produced* as fp32r, so the same bitcast is applied to the producing
    DMA (both sides) or ACT/DVE eviction output."""
    return ap.bitcast(f32r)


def _f8(ap):
    """uint8 DRAM bytes -> fp8e4m3 view (numpy has no fp8 dtype)."""
    return ap.bitcast(f8)


def build_program():
    nc = bacc.Bacc("TRN2", target_bir_lowering=False, debug=False, num_devices=8)

    srcTt = nc.dram_tensor("srcTt", [P, NCH, ET, KCH], bf16, kind="ExternalInput").ap()
    srcq = nc.dram_tensor("srcq", [NQ, E], f32, kind="ExternalInput").ap()
    Wqt = nc.dram_tensor("Wqt", [P, ET, ET, P], bf16, kind="ExternalInput").ap()
    Wkt = nc.dram_tensor("Wkt", [P, ET, ET, P], bf16, kind="ExternalInput").ap()
    Wvt = nc.dram_tensor("Wvt", [P, 2, ET, 512], bf16, kind="ExternalInput").ap()
    Wot = nc.dram_tensor("Wot", [P, ET, E], bf16, kind="ExternalInput").ap()
    W1t = nc.dram_tensor("W1t", [P, NFC, ET, 512], u8, kind="ExternalInput").ap()
    W2t = nc.dram_tensor("W2t", [P, NFC, FCH, E], u8, kind="ExternalInput").ap()
    # bq | b1 pre-tiled host-side to [P, ET+FT] (a naive "(t p) -> p t"
    # rearrange DMA costs ~1k 4-byte descriptors).  bk is dropped: it only
    # adds a per-query constant to the logits, which softmax cancels.  bv
    # is folded into the residual host-side as bv@Wo (softmax weights sum
    # to 1).  The LN1 affine is folded into the FF weights host-side
    # (W1' = diag(g1) @ W1, b1' = b1 + be1 @ W1), so the kernel only
    # normalizes x and the transposed copy xT stays affine-free.
    bqb1 = nc.dram_tensor("bqb1", [P, ET + FT], f32, kind="ExternalInput").ap()
    beb2 = nc.dram_tensor("beb2", [E], f32, kind="ExternalInput").ap()
    g1 = nc.dram_tensor("g1", [E], f32, kind="ExternalInput").ap()
    g2 = nc.dram_tensor("g2", [E], f32, kind="ExternalInput").ap()
    be2 = nc.dram_tensor("be2", [E], f32, kind="ExternalInput").ap()
    out = nc.dram_tensor("out", [NQ, E], f32, kind="ExternalOutput").ap()

    with tile.TileContext(nc) as tc, ExitStack() as ctx:
        consts = ctx.enter_context(tc.tile_pool(name="consts", bufs=1))
        lnp = ctx.enter_context(tc.tile_pool(name="lnp", bufs=4))

        def bcast(vec, n, pool):
            t = pool.tile([P, n], f32, tag=f"bc_{vec.tensor.name}")
            nc.sync.dma_start(out=t, in_=vec.partition_broadcast(P))
            return t

        def ln_stats(t):
            """mean/rstd of t along the free dim -> (mv, rstd)."""
            stats = lnp.tile([P, 2, 6], f32, tag="stats")
            for sg in range(2):
                nc.vector.bn_stats(out=stats[:, sg, :], in_=t[:, sg * 512:(sg + 1) * 512])
            mv = lnp.tile([P, 2], f32, tag="mv")
            nc.vector.bn_aggr(out=mv, in_=stats)
            rstd = lnp.tile([P, 1], f32, tag="rstd")
            nc.scalar.activation(out=rstd, in_=mv[:, 1:2],
                                 func=mybir.ActivationFunctionType.Sqrt,
                                 bias=eps_sb, scale=1.0)
            nc.vector.reciprocal(out=rstd, in_=rstd)
            return mv, rstd

        def ln_normalize(t):
            """t = (t - mu) * rstd, one fused Vector op (no affine)."""
            mv, rstd = ln_stats(t)
            nc.vector.tensor_scalar(out=t, in0=t, scalar1=mv[:, 0:1],
                                    scalar2=rstd, op0=SUB, op1=MULT)

        def layernorm_inplace(t, g_bc, be_bc):
            """t: [P, E] SBUF tile; LN along free dim, then *g + be."""
            mv, rstd = ln_stats(t)
            nc.vector.scalar_tensor_tensor(out=t, in0=t, scalar=mv[:, 0:1],
                                           in1=g_bc, op0=SUB, op1=MULT)
            nc.vector.scalar_tensor_tensor(out=t, in0=t, scalar=rstd,
                                           in1=be_bc, op0=MULT, op1=ADD)

        stA = ExitStack()   # aT lives through phase 3
        late = ExitStack()  # pools allocated after phase 2 frees its SBUF
        try:
            aT_pool = stA.enter_context(tc.tile_pool(name="aT_pool", bufs=1))
            aT = aT_pool.tile([P, ET, NQ], bf16)

            # ------------- Phase 1+2: QKV projections + attention -------------
            with ExitStack() as ph2:
                qT_pool = ph2.enter_context(tc.tile_pool(name="qT_pool", bufs=1))
                qT = qT_pool.tile([P, ET, NQ], bf16)
                stpc_pool = ph2.enter_context(tc.tile_pool(name="stpc", bufs=3))
                kc_pool = ph2.enter_context(tc.tile_pool(name="kcp", bufs=2))
                vc_pool = ph2.enter_context(tc.tile_pool(name="vcp", bufs=2))
                wk_pool = ph2.enter_context(tc.tile_pool(name="wk", bufs=10))
                wv_pool = ph2.enter_context(tc.tile_pool(name="wv", bufs=2))
                ps_kv = ph2.enter_context(tc.tile_pool(name="ps_kv", bufs=2, space="PSUM"))
                ps_s = ph2.enter_context(tc.tile_pool(name="ps_s", bufs=2, space="PSUM"))
                ps_a = ph2.enter_context(tc.tile_pool(name="ps_a", bufs=2, space="PSUM"))
                ps_sum = ph2.enter_context(tc.tile_pool(name="ps_sum", bufs=1, space="PSUM"))

                # per-partition bias layouts: element i lands at [i % 128, i // 128]
                bias_sb = consts.tile([P, ET + FT], f32)
                nc.sync.dma_start(out=bias_sb, in_=bqb1)
                bq_sb = bias_sb[:, 0:ET]
                b1_sb = bias_sb[:, ET:]

                ones0 = consts.tile([P, 1], f32)
                nc.vector.memset(ones0, 1.0)
                ones_sb = consts.tile([P, 1], f32)
                nc.vector.tensor_copy(out=_mm(ones_sb), in_=ones0)
                eps_sb = consts.tile([P, 1], f32)
                nc.vector.memset(eps_sb, EPS)
                inv_w2s = consts.tile([P, 1], f32)
                nc.vector.memset(inv_w2s, 1.0 / W2_SCALE)

                sums = []
                for qs in range(QS):
                    sums_t = ps_sum.tile([1, 512], f32, tag=f"sums{qs}", name=f"sums{qs}")
                    sums.append(sums_t)

                kcs, vcs = {}, {}

                def attend(cc, first, last):
                    """S^T -> exp -> sums and aT accumulation for chunk cc."""
                    kc, vc = kcs.pop(cc), vcs.pop(cc)
                    pc = stpc_pool.tile([P, KT, NQ], f32, tag="stpc", name=f"pc{cc}")
                    for kt in range(KT):
                        for qs in range(QS):
                            ps = ps_s.tile([P, 512], f32, tag="ps")
                            for e_t in range(ET):
                                nc.tensor.matmul(ps, kc[:, e_t, kt * P:(kt + 1) * P],
                                                 qT[:, e_t, qs * 512:(qs + 1) * 512],
                                                 start=(e_t == 0), stop=(e_t == ET - 1))
                            nc.scalar.activation(out=_mm(pc[:, kt, qs * 512:(qs + 1) * 512]),
                                                 in_=ps,
                                                 func=mybir.ActivationFunctionType.Exp)
                            nc.tensor.matmul(sums[qs], _mm(ones_sb),
                                             _mm(pc[:, kt, qs * 512:(qs + 1) * 512]),
                                             start=(first and kt == 0),
                                             stop=(last and kt == KT - 1))
                    # aT += vc.T @ pc
                    for qs in range(QS):
                        for e_t in range(ET):
                            ps = ps_a.tile([P, 512], f32, tag="ps")
                            for kt in range(KT):
                                nc.tensor.matmul(ps, _mm(vc[:, kt, e_t * P:(e_t + 1) * P]),
                                                 _mm(pc[:, kt, qs * 512:(qs + 1) * 512]),
                                                 start=(kt == 0), stop=(kt == KT - 1))
                            dst = aT[:, e_t, qs * 512:(qs + 1) * 512]
                            if first:
                                nc.vector.tensor_copy(out=dst, in_=ps)
                            else:
                                nc.vector.tensor_add(out=dst, in0=dst, in1=ps)

                sts = {}

                def st_load(cc, split=False):
                    st = stpc_pool.tile([P, ET, KCH], bf16, tag="stpc", name=f"st{cc}")
                    if split:
                        # per-d_t transfers: the first projection matmul only
                        # waits on slice 0, not the whole 2MB chunk
                        for d_t in range(ET):
                            nc.sync.dma_start(out=st[:, d_t, :],
                                              in_=srcTt[:, cc, d_t, :])
                    else:
                        nc.sync.dma_start(out=st, in_=srcTt[:, cc])
                    sts[cc] = st

                def kproj(pr, c0):
                    """K^T chunks [e, kc] for the pair, one Wk tile pass."""
                    for cc in (c0, c0 + 1):
                        kcs[cc] = kc_pool.tile([P, ET, KCH], bf16, tag="kc", name=f"kc{cc}")
                    for e_t in range(ET):
                        wk_t = wk_pool.tile([P, ET, P], bf16, tag="wk", name=f"wk{pr}_{e_t}")
                        nc.sync.dma_start(out=wk_t, in_=Wkt[:, e_t])
                        for cc in (c0, c0 + 1):
                            ps = ps_kv.tile([P, KCH], f32, tag="ps")
                            for d_t in range(ET):
                                nc.tensor.matmul(ps, wk_t[:, d_t, :],
                                                 sts[cc][:, d_t, :],
                                                 start=(d_t == 0), stop=(d_t == ET - 1))
                            nc.vector.tensor_copy(out=kcs[cc][:, e_t, :], in_=ps)

                def vproj(pr, c0):
                    """V chunks [kc, e] for the pair, one Wv tile pass."""
                    for cc in (c0, c0 + 1):
                        vcs[cc] = vc_pool.tile([P, KT, E], f32, tag="vc", name=f"vc{cc}")
                    for es in range(2):
                        wv_t = wv_pool.tile([P, ET, 512], bf16, tag="wv", name=f"wv{pr}_{es}")
                        nc.sync.dma_start(out=wv_t, in_=Wvt[:, es])
                        for cc in (c0, c0 + 1):
                            for kt in range(KT):
                                ps = ps_kv.tile([P, 512], f32, tag="ps")
                                for d_t in range(ET):
                                    nc.tensor.matmul(
                                        ps, sts[cc][:, d_t, kt * P:(kt + 1) * P],
                                        wv_t[:, d_t, :],
                                        start=(d_t == 0), stop=(d_t == ET - 1))
                                nc.vector.tensor_copy(
                                    out=_mm(vcs[cc][:, kt, es * 512:(es + 1) * 512]),
                                    in_=ps)

                # -- pair 0, startup-critical DMA order: wq0-slice, st0,
                # wq0-rest, wq1..7, st1 -- all Wq is in flight before st1 so
                # the Q matmul stream never outruns its weights
                wq_ts = [wk_pool.tile([P, ET, P], bf16, tag="wk", name="wq0")]
                nc.sync.dma_start(out=wq_ts[0][:, 0, :], in_=Wqt[:, 0, 0, :])
                st_load(0, split=True)
                for d_t in range(1, ET):
                    nc.sync.dma_start(out=wq_ts[0][:, d_t, :],
                                      in_=Wqt[:, 0, d_t, :])
                for e_t in range(1, ET):
                    nxt = wk_pool.tile([P, ET, P], bf16, tag="wk", name=f"wq{e_t}")
                    nc.sync.dma_start(out=nxt, in_=Wqt[:, e_t])
                    wq_ts.append(nxt)
                st_load(1, split=True)
                # Q projection for both query slices (srcT is permuted so
                # chunks 0-1 ARE the core's query rows); one Wq tile pass.
                # (e_t=0, qs=1) is deferred one iteration so the first matmuls
                # only need st0 -- st1 is still streaming in at that point.
                def qproj(e_t, qs):
                    ps = ps_kv.tile([P, 512], f32, tag="ps")
                    for d_t in range(ET):
                        nc.tensor.matmul(ps, wq_ts[e_t][:, d_t, :],
                                         sts[qs][:, d_t, :],
                                         start=(d_t == 0), stop=(d_t == ET - 1))
                    nc.vector.tensor_scalar_add(
                        out=qT[:, e_t, qs * 512:(qs + 1) * 512],
                        in0=ps, scalar1=bq_sb[:, e_t:e_t + 1])

                for e_t in range(ET):
                    qproj(e_t, 0)
                    if e_t == 1:
                        qproj(0, 1)
                    if e_t >= 1:
                        qproj(e_t, 1)
                kproj(0, 0)
                vproj(0, 0)
                # interleave pair-1 DMAs/projections between the pair-0
                # attends so pair-1's weight traffic hides under compute
                attend(0, first=True, last=False)
                st_load(2)
                st_load(3)
                kproj(1, 2)
                attend(1, first=False, last=False)
                vproj(1, 2)
                attend(2, first=False, last=False)
                attend(3, first=False, last=True)
                del sts

                # softmax denominators: spread sums[1, q] across partitions
                # via K=1 matmuls (1-partition DMAs fail NEFF load)
                sums_sb = consts.tile([1, NQ], f32)
                for qs in range(QS):
                    nc.vector.tensor_copy(out=sums_sb[:, qs * 512:(qs + 1) * 512],
                                          in_=sums[qs])
                one_sp = consts.tile([1, 1], f32)
                nc.vector.memset(one_sp, 1.0)
                rsum = consts.tile([P, ET], f32)
                for t in range(ET):
                    pst = ps_kv.tile([P, 1], f32, tag="ps", name=f"spread{t}")
                    nc.tensor.matmul(pst, sums_sb[0:1, t * P:(t + 1) * P], one_sp,
                                     start=True, stop=True)
                    nc.vector.tensor_copy(out=rsum[:, t:t + 1], in_=pst)
                nc.vector.reciprocal(out=rsum, in_=rsum)

            # ------------- Phase 3: O, residual, LN1, transpose -------------
            # wo first: it lands in the earliest-freed phase-2 SBUF, so its
            # DMA can start before the attention tail fully drains
            wo_pool = late.enter_context(tc.tile_pool(name="wo", bufs=1))
            x_pool = late.enter_context(tc.tile_pool(name="x_pool", bufs=1))
            xT_pool = late.enter_context(tc.tile_pool(name="xT_pool", bufs=1))
            wc_pool = late.enter_context(tc.tile_pool(name="wc", bufs=12))
            bcp = late.enter_context(tc.tile_pool(name="bcp", bufs=1))
            x_sb = x_pool.tile([P, ET, E], f32)   # [q(8x128), e]
            xT = xT_pool.tile([P, ET, NQ], f8)    # [e, q] fp8 for the FF

            g1_bc = bcast(g1, E, bcp)
            beb2_bc = bcast(beb2, E, bcp)   # be1 + b2, combined host-side
            ident = bcp.tile([P, P], f32, tag="ident")
            make_identity(nc, ident)

            # per-tile DMAs so the first O matmul only waits on wo[0]
            wo_sb = wo_pool.tile([P, ET, E], bf16)
            for e_t in range(ET):
                nc.sync.dma_start(out=wo_sb[:, e_t, :], in_=Wot[:, e_t])

            with ExitStack() as ph3:
                sq2_pool = ph3.enter_context(tc.tile_pool(name="sq2", bufs=3))
                ps_o = ph3.enter_context(tc.tile_pool(name="ps_o", bufs=6, space="PSUM"))
                ps_t = ph3.enter_context(tc.tile_pool(name="ps_t", bufs=2, space="PSUM"))

                # prefetch first FF chunk weights during phase 3
                w1c0 = wc_pool.tile([P, ET, 512], f8, tag="wc", name="w1c0")
                nc.sync.dma_start(out=w1c0, in_=_f8(W1t[:, 0]))
                w2c0 = wc_pool.tile([P, FCH, E], f8, tag="wc", name="w2c0")
                nc.sync.dma_start(out=w2c0, in_=_f8(W2t[:, 0]))

                for q_t in range(ET):
                    sq = sq2_pool.tile([P, E], f32, tag="sq")
                    nc.sync.dma_start(out=sq, in_=srcq[q_t * P:(q_t + 1) * P, :])
                    xt_row = x_sb[:, q_t, :]
                    for eo in range(2):
                        ps = ps_o.tile([P, 512], f32, tag="ps")
                        for e_t in range(ET):
                            nc.tensor.matmul(ps, aT[:, e_t, q_t * P:(q_t + 1) * P],
                                             wo_sb[:, e_t, eo * 512:(eo + 1) * 512],
                                             start=(e_t == 0), stop=(e_t == ET - 1))
                        # x = O*rsum + (src+bo), one fused Vector op
                        # (bo and bv@Wo are folded into srcq on the host)
                        nc.vector.scalar_tensor_tensor(
                            out=x_sb[:, q_t, eo * 512:(eo + 1) * 512], in0=ps,
                            scalar=rsum[:, q_t:q_t + 1],
                            in1=sq[:, eo * 512:(eo + 1) * 512], op0=MULT, op1=ADD)
                    # normalize only -- the g1/be1 affine rides the transpose
                    # eviction (per-partition in the transposed domain)
                    ln_normalize(xt_row)
                    for e_t in range(ET):
                        pst = ps_t.tile([P, P], f32, tag="ps")
                        nc.tensor.transpose(pst, x_sb[:, q_t, e_t * P:(e_t + 1) * P], ident)
                        nc.scalar.activation(out=xT[:, e_t, q_t * P:(q_t + 1) * P],
                                             in_=pst,
                                             func=mybir.ActivationFunctionType.Copy)

            # ------------- Phase 4: feedforward (fp8 DoubleRow) + LN2 -------
            g2_bc = bcast(g2, E, bcp)
            be2_bc = bcast(be2, E, bcp)
            with ExitStack() as ph4:
                # x_sb holds normalized-but-unaffined LN1 output; rebuild the
                # residual base x*g1 + (be1+b2) here, in group-0's Vector slack
                for q_t in range(ET):
                    row = x_sb[:, q_t, :]
                    nc.vector.tensor_mul(out=row, in0=row, in1=g1_bc)
                    nc.vector.tensor_add(out=row, in0=row, in1=beb2_bc)
                hc_pool = ph4.enter_context(tc.tile_pool(name="hc", bufs=6))
                sqs_pool = ph4.enter_context(tc.tile_pool(name="sqs", bufs=2))
                ps_h = ph4.enter_context(tc.tile_pool(name="ps_h", bufs=2, space="PSUM"))
                ps_f = ph4.enter_context(tc.tile_pool(name="ps_f", bufs=6, space="PSUM"))

                # FF chunk groups: one PSUM accumulation per group; the
                # large last group gives the inline LN2 a window to hide in
                GRPS = [2, 6]
                for g, GRP in enumerate(GRPS):
                    gbase = sum(GRPS[:g])
                    w1cs, w2cs, hTcs = [], [], []
                    for h in range(GRP):
                        fc = gbase + h
                        if fc == 0:
                            w1c = w1c0
                        else:
                            w1c = wc_pool.tile([P, ET, 512], f8, tag="wc", name=f"w1c{fc}")
                            nc.sync.dma_start(out=w1c, in_=_f8(W1t[:, fc]))
                        hTc = hc_pool.tile([P, FCH, NQ], f8, tag="hc", name=f"hc{fc}")
                        w1cs.append(w1c)
                        hTcs.append(hTc)
                        for fl in range(FCH):
                            f_t = fc * FCH + fl
                            for qs in range(QS):
                                ps = ps_h.tile([P, 512], f32, tag="ps")
                                for dp in range(ET // 2):
                                    nc.tensor.matmul(
                                        ps, w1c[:, 2 * dp:2 * dp + 2, fl * P:(fl + 1) * P],
                                        xT[:, 2 * dp:2 * dp + 2, qs * 512:(qs + 1) * 512],
                                        start=(dp == 0), stop=(dp == ET // 2 - 1),
                                        perf_mode=DR)
                                # h = relu(ps/W1_SCALE + b1), cast to fp8
                                nc.scalar.activation(
                                    out=hTc[:, fl, qs * 512:(qs + 1) * 512], in_=ps,
                                    func=mybir.ActivationFunctionType.Relu,
                                    bias=b1_sb[:, f_t:f_t + 1], scale=1.0 / W1_SCALE)

                        fc2 = gbase + h
                        if fc2 == 0:
                            w2cs.append(w2c0)
                        else:
                            w2c = wc_pool.tile([P, FCH, E], f8, tag="wc", name=f"w2c{fc2}")
                            nc.sync.dma_start(out=w2c, in_=_f8(W2t[:, fc2]))
                            w2cs.append(w2c)

                    last = g == len(GRPS) - 1
                    for q_t in range(ET):
                        asum = (lnp.tile([P, 4], f32, tag="asum", name=f"asum{q_t}")
                                if last else None)
                        for eo in range(2):
                            ps = ps_f.tile([P, 512], f32, tag="ps")
                            for h in range(GRP):
                                for fp_ in range(FCH // 2):
                                    nc.tensor.matmul(
                                        ps,
                                        hTcs[h][:, 2 * fp_:2 * fp_ + 2, q_t * P:(q_t + 1) * P],
                                        w2cs[h][:, 2 * fp_:2 * fp_ + 2, eo * 512:(eo + 1) * 512],
                                        start=(h == 0 and fp_ == 0),
                                        stop=(h == GRP - 1 and fp_ == FCH // 2 - 1),
                                        perf_mode=DR)
                            dst = x_sb[:, q_t, eo * 512:(eo + 1) * 512]
                            # x += ps/W2_SCALE; in the last group the same op
                            # emits sum(x2) per half as LN2's mean input
                            nc.vector.scalar_tensor_tensor(
                                out=dst, in0=ps, scalar=inv_w2s, in1=dst,
                                op0=MULT, op1=ADD,
                                accum_out=asum[:, eo:eo + 1] if last else None)
                            if last:
                                # sum(x2^2) via an idle-Scalar Square pass
                                sqscr = sqs_pool.tile([P, 512], f32, tag="sq")
                                nc.scalar.activation(
                                    out=sqscr, in_=dst,
                                    func=mybir.ActivationFunctionType.Square,
                                    accum_out=asum[:, 2 + eo:3 + eo])
                        if last:
                            # final row: LN2 from the accumulated sums + store
                            # -- overlaps the remaining rows' matmuls
                            row = x_sb[:, q_t, :]
                            mu = lnp.tile([P, 1], f32, tag="mu1")
                            nc.vector.tensor_add(out=mu, in0=asum[:, 0:1],
                                                 in1=asum[:, 1:2])
                            nc.vector.tensor_scalar_mul(out=mu, in0=mu,
                                                        scalar1=1.0 / E)
                            vr = lnp.tile([P, 1], f32, tag="vr")
                            nc.vector.tensor_add(out=vr, in0=asum[:, 2:3],
                                                 in1=asum[:, 3:4])
                            mu2 = lnp.tile([P, 1], f32, tag="mu2")
                            nc.vector.tensor_mul(out=mu2, in0=mu, in1=mu)
                            nc.vector.tensor_scalar(out=vr, in0=vr,
                                                    scalar1=1.0 / E, scalar2=mu2,
                                                    op0=MULT, op1=SUB)
                            rstd = lnp.tile([P, 1], f32, tag="rstd2")
                            nc.scalar.activation(
                                out=rstd, in_=vr,
                                func=mybir.ActivationFunctionType.Sqrt,
                                bias=eps_sb, scale=1.0)
                            nc.vector.reciprocal(out=rstd, in_=rstd)
                            nc.vector.scalar_tensor_tensor(out=row, in0=row,
                                                           scalar=mu, in1=g2_bc,
                                                           op0=SUB, op1=MULT)
                            nc.vector.scalar_tensor_tensor(out=row, in0=row,
                                                           scalar=rstd, in1=be2_bc,
                                                           op0=MULT, op1=ADD)
                            nc.sync.dma_start(out=out[q_t * P:(q_t + 1) * P, :], in_=row)
        finally:
            # LIFO: `late` pools were entered after stA, so release them first
            late.close()
            stA.close()

    nc.compile()
    return nc


_NC_CACHE = None


def _pretile(inputs):
    """Host-side re-layouts so every DMA is contiguous per partition.
    W1/W2 are scaled and quantized to fp8 e4m3 (passed as uint8 bytes)."""
    import ml_dtypes
    e4m3 = ml_dtypes.float8_e4m3fn
    Wq = np.asarray(inputs["Wq"], np.float32)
    Wk = np.asarray(inputs["Wk"], np.float32)
    Wv = np.asarray(inputs["Wv"], np.float32)
    Wo = np.asarray(inputs["Wo"], np.float32)
    # fold the LN1 affine into the first FF layer (exact):
    #   (g1*x^ + be1) @ W1 + b1  ==  x^ @ (diag(g1) @ W1) + (b1 + be1 @ W1)
    W1raw = np.asarray(inputs["W1"], np.float32)
    g1v = np.asarray(inputs["g1"], np.float32)
    be1v = np.asarray(inputs["be1"], np.float32)
    W1 = (g1v[:, None] * W1raw) * W1_SCALE
    b1f = np.asarray(inputs["b1"], np.float32) + be1v @ W1raw
    W2 = np.asarray(inputs["W2"], np.float32) * W2_SCALE
    c = np.ascontiguousarray
    bf = ml_dtypes.bfloat16
    d = {
        "Wqt": c(Wq.reshape(ET, P, ET, P).transpose(1, 2, 0, 3).astype(bf)),
        "Wkt": c(Wk.reshape(ET, P, ET, P).transpose(1, 2, 0, 3).astype(bf)),
        "Wvt": c(Wv.reshape(ET, P, 2, 512).transpose(1, 2, 0, 3).astype(bf)),
        "Wot": c(Wo.reshape(ET, P, E).transpose(1, 0, 2).astype(bf)),
        "W1t": c(W1.reshape(ET, P, NFC, 512).transpose(1, 2, 0, 3)
                 .astype(e4m3)).view(np.uint8),
        "W2t": c(W2.reshape(NFC, FCH, P, E).transpose(2, 0, 1, 3)
                 .astype(e4m3)).view(np.uint8),
        "bqb1": c(np.concatenate(
            [np.asarray(inputs["bq"], np.float32).reshape(ET, P).T,
             b1f.reshape(FT, P).T], axis=1)),
        "beb2": c(be1v + np.asarray(inputs["b2"], np.float32)),
    }
    for n in ["g1", "g2", "be2"]:
        d[n] = c(np.asarray(inputs[n], np.float32))
    return d


def make_in_maps(inputs):
    src = np.ascontiguousarray(np.asarray(inputs["src"], dtype=np.float32))
    # residual offset: out-projection bias + bv routed through Wo
    # (attention weights sum to 1, so  softmax(S) @ (V0 + bv) @ Wo + bo
    #  == softmax(S) @ V0 @ Wo + (bv @ Wo + bo))
    bo = (np.asarray(inputs["bo"], np.float32)
          + np.asarray(inputs["bv"], np.float32)
          @ np.asarray(inputs["Wo"], np.float32))
    shared = _pretile(inputs)

    in_maps = []
    for core in range(8):
        b, h = core // 2, core % 2
        src_b = src[b]                        # [2048, 1024]
        # permute context so this core's query half is columns 0..1023
        perm = np.concatenate([src_b[h * NQ:(h + 1) * NQ, :],
                               src_b[(1 - h) * NQ:(2 - h) * NQ, :]])
        srcT = perm.T                         # [1024, 2048]
        # [p, c, t, k] = srcT[t*128+p, c*512+k]; bf16 on the wire (the
        # projection matmuls consume it in bf16)
        import ml_dtypes
        srcTt = np.ascontiguousarray(
            srcT.reshape(ET, P, NCH, KCH).transpose(1, 2, 0, 3)
            .astype(ml_dtypes.bfloat16))
        # bo (the out-projection bias) is folded into the residual here
        srcq = np.ascontiguousarray(src_b[h * NQ:(h + 1) * NQ, :] + bo)
        in_maps.append({"srcTt": srcTt, "srcq": srcq, **shared})
    return in_maps


def gather_out(results):
    out = np.empty((4, S, E), np.float32)
    for core in range(8):
        b, h = core // 2, core % 2
        out[b, h * NQ:(h + 1) * NQ, :] = results[core]["out"]
    return out


def kernel(**inputs):
    global _NC_CACHE
    from concourse.bass_utils import run_bass_kernel_spmd

    in_maps = make_in_maps(inputs)
    if _NC_CACHE is None:
        _NC_CACHE = build_program()
    res = run_bass_kernel_spmd(_NC_CACHE, in_maps, list(range(8)))
    return gather_out(res.results)


if __name__ == "__main__":
    nc = build_program()
    print("build + compile OK")
